# revision 7
# baseline (speedup 1.0000x reference)
"""Trainium2 Bass kernel: fused 1x1conv -> QuantReLU -> 1x1conv -> QuantReLU.

Reference computation (per element, all f32):
    u1 = fq(w1) @ x + fq(b1)           # 1x1 conv == channel GEMM
    h  = clip(round(relu(u1)/s), 0, 255) * s
    u2 = fq(w2) @ h + fq(b2)
    y  = clip(round(relu(u2)/s), 0, 255) * s

Shapes (hardcoded): x [16, 512, 2048] f32, w1/w2 [512, 512], b1/b2 [512],
act_scale [1].  Sharding: data-parallel over batch, 2 batches per core on
8 cores.  No collectives needed.

Device strategy per core:
  - Weights are fake-quantized on host to integer grids q in [-127,127];
    integers <= 255 are exact in bf16, so matmuls run on the fast bf16 path.
  - Layer 1: x is split on host into bf16 hi + bf16 lo (x = hi + lo to
    ~16-bit precision); two bf16 matmul passes accumulate into the same
    PSUM bank, recovering ~f32 GEMM accuracy at 2 cycles/row instead of
    fp32's 4.
  - Layer 2: quantized activations h_int in {0..255} are exact bf16, and
    the f32 PSUM accumulator is exact below 2^24 (max sum 512*127*255 <
    2^24), so a single bf16 pass is bit-exact.
  - Quant-ReLU epilogue is 3 elementwise ops spread over 3 engines:
      ACT:    t = Relu(psum * (s_w/s_act) + b_q/s_act)       (per-chan bias AP)
      DVE:    r = min(t + 2^23, 2^23 + 255)    # f32 add does RNE round
      GPSIMD: h = r - 2^23  (written as bf16; exact for 0..255)
  - Output codes y_int {0..255} leave the device as bf16 (half the DMA
    bytes); the host multiplies by act_scale in f32 (exact).
"""

import numpy as np
import ml_dtypes

import concourse.bass as bass
import concourse.mybir as mybir
from concourse import bacc, tile
from concourse.bass_utils import run_bass_kernel_spmd

BF16 = ml_dtypes.bfloat16

B, C, N = 16, 512, 2048
NCORES = 8
BPC = B // NCORES           # batches per core (2)
P = 128                     # SBUF partitions
KC = C // P                 # contraction chunks (4)
MC = C // P                 # output-channel chunks (4)
NT = 512                    # free-dim tile (one PSUM bank of f32)
NTPB = N // NT              # n-tiles per batch (4)
NTILES = BPC * NTPB         # n-tiles per core (8)
MAGIC = float(2 ** 23)

MODE = "hilo"               # 'hilo': bf16 hi+lo layer-1 (safest) | 'fp16'

AF = mybir.ActivationFunctionType
ALU = mybir.AluOpType
DT = mybir.dt

_NC_CACHE: dict = {}


def _build_nc(mode: str) -> bass.Bass:
    """Build the per-core Bass program (identical on all 8 cores)."""
    # Bacc (not Bass): its compile() legalizes Tile's multi-wait sync_info
    # into single-wait instructions + EventSemaphores, as walrus requires.
    nc = bacc.Bacc("TRN2", target_bir_lowering=False)
    x_dt = DT.bfloat16 if mode == "hilo" else DT.float16
    x_cols = 2 * NT if mode == "hilo" else NT
    n_l1_pass = 2 if mode == "hilo" else 1

    xp = nc.declare_dram_parameter("xp", [NTILES, KC, P, x_cols], x_dt, isOutput=False)
    w1d = nc.declare_dram_parameter("w1d", [KC, P, C], x_dt, isOutput=False)
    w2d = nc.declare_dram_parameter("w2d", [KC, P, C], DT.bfloat16, isOutput=False)
    # consts[p, 0]=s1/s_act  [p,1]=s2  [p,2+mc]=b1q/s_act  [p,6+mc]=b2q/s_act
    cd = nc.declare_dram_parameter("cd", [P, 10], DT.float32, isOutput=False)
    yo = nc.declare_dram_parameter("yo", [NTILES, P, MC * NT], DT.bfloat16, isOutput=True)

    with tile.TileContext(nc) as tc:
        with (
            tc.tile_pool(name="wpool", bufs=1) as wpool,
            tc.tile_pool(name="xpool", bufs=3) as xpool,
            tc.tile_pool(name="hpool", bufs=2) as hpool,
            tc.tile_pool(name="tpool", bufs=3) as tpool,
            tc.tile_pool(name="ypool", bufs=2) as ypool,
            tc.tile_pool(name="pspool", bufs=1, space="PSUM") as pspool,
        ):
            # --- persistent tiles: weights + consts, one DMA each ------------
            w1a = wpool.tile([P, KC * C], x_dt, tag="w1a")
            nc.sync.dma_start(
                out=w1a[:].rearrange("p (k c) -> p k c", k=KC),
                in_=w1d.rearrange("k p c -> p k c"),
            )
            w2a = wpool.tile([P, KC * C], DT.bfloat16, tag="w2a")
            nc.sync.dma_start(
                out=w2a[:].rearrange("p (k c) -> p k c", k=KC),
                in_=w2d.rearrange("k p c -> p k c"),
            )
            ct = wpool.tile([P, 10], DT.float32, tag="ct")
            nc.sync.dma_start(out=ct[:], in_=cd[:])

            def w1_lhsT(kc, mc):
                return w1a[:, kc * C + mc * P: kc * C + (mc + 1) * P]

            def w2_lhsT(kc, mc):
                return w2a[:, kc * C + mc * P: kc * C + (mc + 1) * P]

            # --- software-pipelined main loop: L2 runs one n-tile behind L1 --
            h_store: dict[int, list] = {}
            for nt in range(NTILES + 1):
                if nt < NTILES:
                    xt = xpool.tile([P, KC * x_cols], x_dt, tag="x")
                    nc.sync.dma_start(
                        out=xt[:].rearrange("p (k c) -> p k c", k=KC),
                        in_=xp[nt].rearrange("k p c -> p k c"),
                    )
                    hts = []
                    for mc in range(MC):
                        ps = pspool.tile([P, NT], DT.float32, tag=f"ps1_{mc}")
                        for kc in range(KC):
                            for hl in range(n_l1_pass):
                                off = kc * x_cols + hl * NT
                                nc.tensor.matmul(
                                    ps[:],
                                    w1_lhsT(kc, mc),
                                    xt[:, off: off + NT],
                                    start=(kc == 0 and hl == 0),
                                    stop=(kc == KC - 1 and hl == n_l1_pass - 1),
                                )
                        t = tpool.tile([P, NT], DT.float32, tag="t1")
                        nc.scalar.activation(
                            t[:], ps[:], AF.Relu,
                            bias=ct[:, 2 + mc: 3 + mc], scale=ct[:, 0:1],
                        )
                        r = tpool.tile([P, NT], DT.float32, tag="r1")
                        nc.vector.tensor_scalar(
                            r[:], t[:], MAGIC, MAGIC + 255.0, ALU.add, ALU.min
                        )
                        h = hpool.tile([P, NT], DT.bfloat16, tag=f"h{mc}")
                        nc.gpsimd.tensor_scalar_sub(h[:], r[:], MAGIC)
                        hts.append(h)
                    h_store[nt] = hts
                if nt >= 1:
                    hts = h_store.pop(nt - 1)
                    yt = ypool.tile([P, MC * NT], DT.bfloat16, tag="y")
                    for mc in range(MC):
                        ps = pspool.tile([P, NT], DT.float32, tag=f"ps2_{mc}")
                        for kc in range(KC):
                            nc.tensor.matmul(
                                ps[:],
                                w2_lhsT(kc, mc),
                                hts[kc][:],
                                start=(kc == 0),
                                stop=(kc == KC - 1),
                            )
                        t = tpool.tile([P, NT], DT.float32, tag="t2")
                        nc.scalar.activation(
                            t[:], ps[:], AF.Relu,
                            bias=ct[:, 6 + mc: 7 + mc], scale=ct[:, 1:2],
                        )
                        r = tpool.tile([P, NT], DT.float32, tag="r2")
                        nc.vector.tensor_scalar(
                            r[:], t[:], MAGIC, MAGIC + 255.0, ALU.add, ALU.min
                        )
                        nc.gpsimd.tensor_scalar_sub(
                            yt[:, mc * NT:(mc + 1) * NT], r[:], MAGIC
                        )
                    nc.sync.dma_start(out=yo[nt - 1], in_=yt[:])
    nc.compile()
    return nc


def _po2_scale(t: np.ndarray) -> np.float32:
    # match reference._po2_scale bit-for-bit in float32
    thr = np.maximum(np.max(np.abs(t.astype(np.float32))), np.float32(1e-10))
    return np.float32(np.exp2(np.ceil(np.log2(thr))) / np.float32(128.0))


def _fq_weight_int(w: np.ndarray) -> tuple[np.ndarray, np.float32]:
    s = _po2_scale(w)
    q = np.clip(np.round(w.astype(np.float32) / s), -127.0, 127.0)
    return q, s


def _fq_bias(b: np.ndarray) -> np.ndarray:
    s = _po2_scale(b)
    return np.clip(np.round(b.astype(np.float32) / s), -128.0, 127.0) * s


def _pack_x_core(xc: np.ndarray, mode: str) -> np.ndarray:
    """xc [BPC, C, N] f32 -> [NTILES, KC, P, x_cols] device layout."""
    if mode == "hilo":
        xh = xc.astype(BF16)
        xl = (xc - xh.astype(np.float32)).astype(BF16)
        out = np.empty((BPC, NTPB, KC, P, 2 * NT), BF16)
        out[..., :NT] = xh.reshape(BPC, KC, P, NTPB, NT).transpose(0, 3, 1, 2, 4)
        out[..., NT:] = xl.reshape(BPC, KC, P, NTPB, NT).transpose(0, 3, 1, 2, 4)
        return np.ascontiguousarray(out.reshape(NTILES, KC, P, 2 * NT))
    else:
        x16 = xc.astype(np.float16)
        out = x16.reshape(BPC, KC, P, NTPB, NT).transpose(0, 3, 1, 2, 4)
        return np.ascontiguousarray(out.reshape(NTILES, KC, P, NT))


def _run(inputs: dict, trace: bool = False, mode: str | None = None):
    """Shard, run on 8 cores, unshard.  Returns (y_full, BassKernelResults)."""
    mode = mode or MODE
    x = np.asarray(inputs["x"], dtype=np.float32)
    w1 = np.asarray(inputs["w1"], dtype=np.float32)
    b1 = np.asarray(inputs["b1"], dtype=np.float32)
    w2 = np.asarray(inputs["w2"], dtype=np.float32)
    b2 = np.asarray(inputs["b2"], dtype=np.float32)
    s_act = np.float32(np.asarray(inputs["act_scale"], dtype=np.float32).reshape(-1)[0])

    q1, s1 = _fq_weight_int(w1)
    q2, s2 = _fq_weight_int(w2)
    b1q = _fq_bias(b1)
    b2q = _fq_bias(b2)

    w_dt = BF16 if mode == "hilo" else np.float16
    # lhsT layout: [kc, p, o] with contraction index c = kc*P + p
    w1d = np.ascontiguousarray(q1.T.reshape(KC, P, C).astype(w_dt))
    w2d = np.ascontiguousarray(q2.T.reshape(KC, P, C).astype(BF16))
    cd = np.empty((P, 10), np.float32)
    cd[:, 0] = np.float32(s1 / s_act)
    cd[:, 1] = np.float32(s2)
    cd[:, 2:6] = (b1q / s_act).astype(np.float32).reshape(MC, P).T
    cd[:, 6:10] = (b2q / s_act).astype(np.float32).reshape(MC, P).T

    if mode not in _NC_CACHE:
        _NC_CACHE[mode] = _build_nc(mode)
    nc = _NC_CACHE[mode]

    in_maps = []
    for c in range(NCORES):
        xc = x[c * BPC:(c + 1) * BPC]
        in_maps.append(
            {
                "xp": _pack_x_core(xc, mode),
                "w1d": w1d,
                "w2d": w2d,
                "cd": cd,
            }
        )

    res = run_bass_kernel_spmd(nc, in_maps, core_ids=list(range(NCORES)), trace=trace)

    y = np.empty((B, C, N), np.float32)
    for c in range(NCORES):
        yo = np.asarray(res.results[c]["yo"])  # [NTILES, P, MC*NT] bf16
        yi = (
            yo.astype(np.float32)
            .reshape(BPC, NTPB, P, MC, NT)
            .transpose(0, 3, 2, 1, 4)
            .reshape(BPC, C, N)
        )
        y[c * BPC:(c + 1) * BPC] = yi * s_act
    return y, res


def kernel(**inputs) -> np.ndarray:
    y, _ = _run(inputs, trace=False)
    return y


# revision 8
# speedup vs baseline: 4.7281x; 4.7281x over previous
"""Trainium2 Bass kernel: fused 1x1conv -> QuantReLU -> 1x1conv -> QuantReLU.

Reference computation (per element, all f32):
    u1 = fq(w1) @ x + fq(b1)           # 1x1 conv == channel GEMM
    h  = clip(round(relu(u1)/s), 0, 255) * s
    u2 = fq(w2) @ h + fq(b2)
    y  = clip(round(relu(u2)/s), 0, 255) * s

Shapes (hardcoded): x [16, 512, 2048] f32, w1/w2 [512, 512], b1/b2 [512],
act_scale [1].  Sharding: data-parallel over batch, 2 batches per core on
8 cores.  No collectives needed.

Device strategy per core:
  - Weights are fake-quantized on host to integer grids q in [-127,127];
    small integers are exact in bf16/fp16, so matmuls run the fast 16-bit
    path (1 col/cycle) instead of fp32 (4x slower).
  - Layer 1: x is split on host into bf16 hi + bf16 lo (x = hi + lo gives
    ~16-bit precision); two bf16 matmul passes accumulate into the same
    PSUM bank, recovering ~f32 GEMM accuracy.
  - Quant-ReLU epilogue is exactly 2 elementwise ops (GPSIMD untouched --
    its software elementwise path is ~7us per tile AND it contends for the
    DVE's SBUF ports):
      ACT: t  = Relu(psum * (s_w/s_act) + b_q/s_act)   (per-chan bias AP)
      DVE: h' = fp16((t min 255) + 1024)
    The fp16 write is the rounding step: values in [1024, 1279] sit where
    fp16's ulp is exactly 1, so the f32->fp16 convert performs the
    round-to-nearest-even to the integer grid.  h' = 1024 + h_int exactly.
  - Layer 2 runs on h' directly (fp16 matmul, exact: products < 2^24);
    the +1024 offset is linear, so it folds into the layer-2 bias as
    -1024 * rowsum(q2) * s2 (exact in f32: integer times power-of-two).
  - Output codes y' = 1024 + y_int leave the device as fp16 (half the DMA
    bytes); the host computes (y' - 1024) * act_scale in f32 (exact).
"""

import numpy as np
import ml_dtypes

import concourse.bass as bass
import concourse.mybir as mybir
from concourse import bacc, tile
from concourse.bass_utils import run_bass_kernel_spmd

BF16 = ml_dtypes.bfloat16

B, C, N = 16, 512, 2048
NCORES = 8
BPC = B // NCORES           # batches per core (2)
P = 128                     # SBUF partitions
KC = C // P                 # contraction chunks (4)
MC = C // P                 # output-channel chunks (4)
NT = 512                    # free-dim tile (one PSUM bank of f32)
NTPB = N // NT              # n-tiles per batch (4)
NTILES = BPC * NTPB         # n-tiles per core (8)
OFF = 1024.0                # fp16 rounding offset: [1024, 1280) has ulp 1

MODE = "hilo"               # 'hilo': bf16 hi+lo layer-1 (safest) | 'fp16'

AF = mybir.ActivationFunctionType
ALU = mybir.AluOpType
DT = mybir.dt

_NC_CACHE: dict = {}


def _build_nc(mode: str) -> bass.Bass:
    """Build the per-core Bass program (identical on all 8 cores)."""
    # Bacc (not Bass): its compile() legalizes Tile's multi-wait sync_info
    # into single-wait instructions + EventSemaphores, as walrus requires.
    nc = bacc.Bacc("TRN2", target_bir_lowering=False)
    x_dt = DT.bfloat16 if mode == "hilo" else DT.float16
    x_cols = 2 * NT if mode == "hilo" else NT
    n_l1_pass = 2 if mode == "hilo" else 1

    xp = nc.declare_dram_parameter("xp", [NTILES, KC, P, x_cols], x_dt, isOutput=False)
    w1d = nc.declare_dram_parameter("w1d", [KC, P, C], x_dt, isOutput=False)
    w2d = nc.declare_dram_parameter("w2d", [KC, P, C], DT.float16, isOutput=False)
    # consts[p, 0]=s1/s_act  [p,1]=s2  [p,2+mc]=b1q/s_act
    # [p,6+mc]=b2q/s_act - 1024*rowsum(q2)*s2
    cd = nc.declare_dram_parameter("cd", [P, 10], DT.float32, isOutput=False)
    yo = nc.declare_dram_parameter("yo", [NTILES, P, MC * NT], DT.float16, isOutput=True)

    with tile.TileContext(nc) as tc:
        with (
            tc.tile_pool(name="wpool", bufs=1) as wpool,
            tc.tile_pool(name="xpool", bufs=3) as xpool,
            tc.tile_pool(name="hpool", bufs=2) as hpool,
            tc.tile_pool(name="tpool", bufs=3) as tpool,
            tc.tile_pool(name="ypool", bufs=2) as ypool,
            tc.tile_pool(name="pspool", bufs=1, space="PSUM") as pspool,
        ):
            # --- persistent tiles: weights + consts, one DMA each ------------
            w1a = wpool.tile([P, KC * C], x_dt, tag="w1a")
            nc.sync.dma_start(
                out=w1a[:].rearrange("p (k c) -> p k c", k=KC),
                in_=w1d.rearrange("k p c -> p k c"),
            )
            w2a = wpool.tile([P, KC * C], DT.float16, tag="w2a")
            nc.sync.dma_start(
                out=w2a[:].rearrange("p (k c) -> p k c", k=KC),
                in_=w2d.rearrange("k p c -> p k c"),
            )
            ct = wpool.tile([P, 10], DT.float32, tag="ct")
            nc.sync.dma_start(out=ct[:], in_=cd[:])

            def w1_lhsT(kc, mc):
                return w1a[:, kc * C + mc * P: kc * C + (mc + 1) * P]

            def w2_lhsT(kc, mc):
                return w2a[:, kc * C + mc * P: kc * C + (mc + 1) * P]

            # --- software-pipelined main loop: L2 runs one n-tile behind L1 --
            h_store: dict[int, list] = {}
            for nt in range(NTILES + 1):
                if nt < NTILES:
                    xt = xpool.tile([P, KC * x_cols], x_dt, tag="x")
                    nc.sync.dma_start(
                        out=xt[:].rearrange("p (k c) -> p k c", k=KC),
                        in_=xp[nt].rearrange("k p c -> p k c"),
                    )
                    hts = []
                    for mc in range(MC):
                        ps = pspool.tile([P, NT], DT.float32, tag=f"ps1_{mc}")
                        for kc in range(KC):
                            for hl in range(n_l1_pass):
                                off = kc * x_cols + hl * NT
                                nc.tensor.matmul(
                                    ps[:],
                                    w1_lhsT(kc, mc),
                                    xt[:, off: off + NT],
                                    start=(kc == 0 and hl == 0),
                                    stop=(kc == KC - 1 and hl == n_l1_pass - 1),
                                )
                        t = tpool.tile([P, NT], DT.float32, tag="t1")
                        nc.scalar.activation(
                            t[:], ps[:], AF.Relu,
                            bias=ct[:, 2 + mc: 3 + mc], scale=ct[:, 0:1],
                        )
                        h = hpool.tile([P, NT], DT.float16, tag=f"h{mc}")
                        nc.vector.tensor_scalar(
                            h[:], t[:], 255.0, OFF, ALU.min, ALU.add
                        )
                        hts.append(h)
                    h_store[nt] = hts
                if nt >= 1:
                    hts = h_store.pop(nt - 1)
                    yt = ypool.tile([P, MC * NT], DT.float16, tag="y")
                    for mc in range(MC):
                        ps = pspool.tile([P, NT], DT.float32, tag=f"ps2_{mc}")
                        for kc in range(KC):
                            nc.tensor.matmul(
                                ps[:],
                                w2_lhsT(kc, mc),
                                hts[kc][:],
                                start=(kc == 0),
                                stop=(kc == KC - 1),
                            )
                        t = tpool.tile([P, NT], DT.float32, tag="t2")
                        nc.scalar.activation(
                            t[:], ps[:], AF.Relu,
                            bias=ct[:, 6 + mc: 7 + mc], scale=ct[:, 1:2],
                        )
                        nc.vector.tensor_scalar(
                            yt[:, mc * NT:(mc + 1) * NT], t[:], 255.0, OFF,
                            ALU.min, ALU.add,
                        )
                    nc.sync.dma_start(out=yo[nt - 1], in_=yt[:])
    nc.compile()
    return nc


def _po2_scale(t: np.ndarray) -> np.float32:
    # match reference._po2_scale bit-for-bit in float32
    thr = np.maximum(np.max(np.abs(t.astype(np.float32))), np.float32(1e-10))
    return np.float32(np.exp2(np.ceil(np.log2(thr))) / np.float32(128.0))


def _fq_weight_int(w: np.ndarray) -> tuple[np.ndarray, np.float32]:
    s = _po2_scale(w)
    q = np.clip(np.round(w.astype(np.float32) / s), -127.0, 127.0)
    return q, s


def _fq_bias(b: np.ndarray) -> np.ndarray:
    s = _po2_scale(b)
    return np.clip(np.round(b.astype(np.float32) / s), -128.0, 127.0) * s


def _pack_x_core(xc: np.ndarray, mode: str) -> np.ndarray:
    """xc [BPC, C, N] f32 -> [NTILES, KC, P, x_cols] device layout."""
    if mode == "hilo":
        xh = xc.astype(BF16)
        xl = (xc - xh.astype(np.float32)).astype(BF16)
        out = np.empty((BPC, NTPB, KC, P, 2 * NT), BF16)
        out[..., :NT] = xh.reshape(BPC, KC, P, NTPB, NT).transpose(0, 3, 1, 2, 4)
        out[..., NT:] = xl.reshape(BPC, KC, P, NTPB, NT).transpose(0, 3, 1, 2, 4)
        return np.ascontiguousarray(out.reshape(NTILES, KC, P, 2 * NT))
    else:
        x16 = xc.astype(np.float16)
        out = x16.reshape(BPC, KC, P, NTPB, NT).transpose(0, 3, 1, 2, 4)
        return np.ascontiguousarray(out.reshape(NTILES, KC, P, NT))


def _run(inputs: dict, trace: bool = False, mode: str | None = None):
    """Shard, run on 8 cores, unshard.  Returns (y_full, BassKernelResults)."""
    mode = mode or MODE
    x = np.asarray(inputs["x"], dtype=np.float32)
    w1 = np.asarray(inputs["w1"], dtype=np.float32)
    b1 = np.asarray(inputs["b1"], dtype=np.float32)
    w2 = np.asarray(inputs["w2"], dtype=np.float32)
    b2 = np.asarray(inputs["b2"], dtype=np.float32)
    s_act = np.float32(np.asarray(inputs["act_scale"], dtype=np.float32).reshape(-1)[0])

    q1, s1 = _fq_weight_int(w1)
    q2, s2 = _fq_weight_int(w2)
    b1q = _fq_bias(b1)
    b2q = _fq_bias(b2)

    w_dt = BF16 if mode == "hilo" else np.float16
    # lhsT layout: [kc, p, o] with contraction index c = kc*P + p
    w1d = np.ascontiguousarray(q1.T.reshape(KC, P, C).astype(w_dt))
    w2d = np.ascontiguousarray(q2.T.reshape(KC, P, C).astype(np.float16))
    # layer-2 bias absorbs the +1024 offset on h': exact, since rowsum(q2)
    # is an integer < 2^13 and s2 is a power of two.
    b2_corr = (np.float32(OFF) * q2.sum(axis=1, dtype=np.float64) * np.float64(s2)).astype(
        np.float32
    )
    cd = np.empty((P, 10), np.float32)
    cd[:, 0] = np.float32(s1 / s_act)
    cd[:, 1] = np.float32(s2)
    cd[:, 2:6] = (b1q / s_act).astype(np.float32).reshape(MC, P).T
    cd[:, 6:10] = ((b2q / s_act).astype(np.float32) - b2_corr).reshape(MC, P).T

    if mode not in _NC_CACHE:
        _NC_CACHE[mode] = _build_nc(mode)
    nc = _NC_CACHE[mode]

    in_maps = []
    for c in range(NCORES):
        xc = x[c * BPC:(c + 1) * BPC]
        in_maps.append(
            {
                "xp": _pack_x_core(xc, mode),
                "w1d": w1d,
                "w2d": w2d,
                "cd": cd,
            }
        )

    res = run_bass_kernel_spmd(nc, in_maps, core_ids=list(range(NCORES)), trace=trace)

    y = np.empty((B, C, N), np.float32)
    for c in range(NCORES):
        yo = np.asarray(res.results[c]["yo"])  # [NTILES, P, MC*NT] fp16
        yi = (
            (yo.astype(np.float32) - np.float32(OFF))
            .reshape(BPC, NTPB, P, MC, NT)
            .transpose(0, 3, 2, 1, 4)
            .reshape(BPC, C, N)
        )
        y[c * BPC:(c + 1) * BPC] = yi * s_act
    return y, res


def kernel(**inputs) -> np.ndarray:
    y, _ = _run(inputs, trace=False)
    return y


# revision 9
# speedup vs baseline: 4.9333x; 1.0434x over previous
"""Trainium2 Bass kernel: fused 1x1conv -> QuantReLU -> 1x1conv -> QuantReLU.

Reference computation (per element, all f32):
    u1 = fq(w1) @ x + fq(b1)           # 1x1 conv == channel GEMM
    h  = clip(round(relu(u1)/s), 0, 255) * s
    u2 = fq(w2) @ h + fq(b2)
    y  = clip(round(relu(u2)/s), 0, 255) * s

Shapes (hardcoded): x [16, 512, 2048] f32, w1/w2 [512, 512], b1/b2 [512],
act_scale [1].  Sharding: data-parallel over batch, 2 batches per core on
8 cores.  No collectives needed.

Device strategy per core:
  - Weights are fake-quantized on host to integer grids q in [-127,127];
    small integers are exact in bf16/fp16, so matmuls run the fast 16-bit
    path (1 col/cycle) instead of fp32 (4x slower).
  - Layer 1 'hilo': x is split on host into bf16 hi + bf16 lo (x = hi+lo
    gives ~16-bit precision); two bf16 matmul passes accumulate into the
    same PSUM bank, recovering ~f32 GEMM accuracy.  'fp16': single fp16
    pass, ~6e-3 rel err instead of ~1e-3 but 2/3 the matmul work.
  - Quant-ReLU epilogue is exactly 2 elementwise ops (GPSIMD untouched --
    its software elementwise path is ~7us per tile AND it contends for the
    DVE's SBUF ports):
      op1 (ACT or DVE): t' = psum * (s_w/s_act) + (b_q/s_act + 1024)
      op2 (DVE):        h' = fp16((t' max 1024) min 1279)
    The fp16 write is the rounding step: values in [1024, 1280) sit where
    fp16's ulp is exactly 1, so the f32->fp16 convert performs the
    round-to-nearest-even onto the integer grid.  h' = 1024 + h_int
    exactly; max/min provide the ReLU clip at 0 and the uint8 clip at 255.
  - Layer 2 runs on h' directly (fp16 matmul, exact: all partial sums are
    integers < 2^24); the +1024 offset is linear, so it folds into the
    layer-2 bias as -1024 * rowsum(q2) * s2 (exact: integer * power-of-2).
  - Output codes y' = 1024 + y_int leave the device as fp16 (half the DMA
    bytes); the host computes (y' - 1024) * act_scale in f32 (exact).
  - Loop structure: software-pipelined at m-chunk granularity -- layer-2
    chunks of n-tile nt-1 are interleaved between layer-1 chunks of
    n-tile nt, so the tensor engine never waits on the epilogue chain and
    the kernel tail is one m-chunk, not one full n-tile.
"""

import numpy as np
import ml_dtypes

import concourse.bass as bass
import concourse.mybir as mybir
from concourse import bacc, tile
from concourse.bass_utils import run_bass_kernel_spmd

BF16 = ml_dtypes.bfloat16

B, C, N = 16, 512, 2048
NCORES = 8
BPC = B // NCORES           # batches per core (2)
P = 128                     # SBUF partitions
KC = C // P                 # contraction chunks (4)
MC = C // P                 # output-channel chunks (4)
NT = 512                    # free-dim tile (one PSUM bank of f32)
NTPB = N // NT              # n-tiles per batch (4)
NTILES = BPC * NTPB         # n-tiles per core (8)
OFF = 1024.0                # fp16 rounding offset: [1024, 1280) has ulp 1

MODE = "hilo"               # 'hilo': bf16 hi+lo layer-1 (safest) | 'fp16'

AF = mybir.ActivationFunctionType
ALU = mybir.AluOpType
DT = mybir.dt

_NC_CACHE: dict = {}


def _build_nc(mode: str) -> bass.Bass:
    """Build the per-core Bass program (identical on all 8 cores)."""
    # Bacc (not Bass): its compile() legalizes Tile's multi-wait sync_info
    # into single-wait instructions + EventSemaphores, as walrus requires.
    nc = bacc.Bacc("TRN2", target_bir_lowering=False)
    x_dt = DT.bfloat16 if mode == "hilo" else DT.float16
    x_cols = 2 * NT if mode == "hilo" else NT
    n_l1_pass = 2 if mode == "hilo" else 1

    xp = nc.declare_dram_parameter("xp", [NTILES, KC, P, x_cols], x_dt, isOutput=False)
    w1d = nc.declare_dram_parameter("w1d", [KC, P, C], x_dt, isOutput=False)
    w2d = nc.declare_dram_parameter("w2d", [KC, P, C], DT.float16, isOutput=False)
    # consts[p, 0]=s1/s_act  [p,1]=s2  [p,2+mc]=b1q/s_act + 1024
    # [p,6+mc]=b2q/s_act - 1024*rowsum(q2)*s2 + 1024
    cd = nc.declare_dram_parameter("cd", [P, 10], DT.float32, isOutput=False)
    yo = nc.declare_dram_parameter("yo", [NTILES, P, MC * NT], DT.float16, isOutput=True)

    with tile.TileContext(nc) as tc:
        with (
            tc.tile_pool(name="wpool", bufs=1) as wpool,
            tc.tile_pool(name="xpool", bufs=3) as xpool,
            tc.tile_pool(name="hpool", bufs=2) as hpool,
            tc.tile_pool(name="tpool", bufs=4) as tpool,
            tc.tile_pool(name="ypool", bufs=3) as ypool,
            tc.tile_pool(name="pspool", bufs=1, space="PSUM") as pspool,
        ):
            # --- persistent tiles; interleave w1/x0 DMAs so the first
            # matmul's dependencies (w1[0], x0[0]) land first ---------------
            ct = wpool.tile([P, 10], DT.float32, tag="ct")
            nc.sync.dma_start(out=ct[:], in_=cd[:])
            w1ts, w2ts = [], []
            x_first = []
            for kc in range(KC):
                w1t = wpool.tile([P, C], x_dt, tag=f"w1_{kc}")
                nc.sync.dma_start(out=w1t[:], in_=w1d[kc])
                w1ts.append(w1t)
                xt = xpool.tile([P, x_cols], x_dt, tag=f"x{kc}")
                nc.sync.dma_start(out=xt[:], in_=xp[0, kc])
                x_first.append(xt)
            for kc in range(KC):
                w2t = wpool.tile([P, C], DT.float16, tag=f"w2_{kc}")
                nc.sync.dma_start(out=w2t[:], in_=w2d[kc])
                w2ts.append(w2t)

            def epilogue(ps, layer, mc, out_ap, use_dve_op1):
                """t' = ps*A + B'; out = fp16((t' max 1024) min 1279)."""
                scale_ap = ct[:, layer - 1: layer]
                bias_col = 2 + mc if layer == 1 else 6 + mc
                bias_ap = ct[:, bias_col: bias_col + 1]
                t = tpool.tile([P, NT], DT.float32, tag="t")
                if use_dve_op1:
                    nc.vector.tensor_scalar(
                        t[:], ps[:], scale_ap, bias_ap, ALU.mult, ALU.add
                    )
                else:
                    nc.scalar.activation(
                        t[:], ps[:], AF.Identity, bias=bias_ap, scale=scale_ap
                    )
                nc.vector.tensor_scalar(
                    out_ap, t[:], OFF, OFF + 255.0, ALU.max, ALU.min
                )

            # --- main loop: m-chunk-interleaved software pipeline -----------
            h_store: dict[int, list] = {}
            for nt in range(NTILES + 1):
                xts = None
                if nt < NTILES:
                    if nt == 0:
                        xts = x_first
                    else:
                        xts = []
                        for kc in range(KC):
                            xt = xpool.tile([P, x_cols], x_dt, tag=f"x{kc}")
                            nc.sync.dma_start(out=xt[:], in_=xp[nt, kc])
                            xts.append(xt)
                    h_store[nt] = []
                for mc in range(MC):
                    if nt < NTILES:
                        # layer-1 chunk mc of n-tile nt
                        ps = pspool.tile([P, NT], DT.float32, tag=f"ps1_{mc}")
                        for kc in range(KC):
                            for hl in range(n_l1_pass):
                                nc.tensor.matmul(
                                    ps[:],
                                    w1ts[kc][:, mc * P:(mc + 1) * P],
                                    xts[kc][:, hl * NT:(hl + 1) * NT],
                                    start=(kc == 0 and hl == 0),
                                    stop=(kc == KC - 1 and hl == n_l1_pass - 1),
                                )
                        h = hpool.tile([P, NT], DT.float16, tag=f"h{mc}")
                        epilogue(
                            ps, 1, mc, h[:],
                            use_dve_op1=(mode == "fp16" and mc == 1),
                        )
                        h_store[nt].append(h)
                    if nt >= 1:
                        # layer-2 chunk mc of n-tile nt-1
                        hts = h_store[nt - 1]
                        ps = pspool.tile([P, NT], DT.float32, tag=f"ps2_{mc}")
                        for kc in range(KC):
                            nc.tensor.matmul(
                                ps[:],
                                w2ts[kc][:, mc * P:(mc + 1) * P],
                                hts[kc][:],
                                start=(kc == 0),
                                stop=(kc == KC - 1),
                            )
                        yt = ypool.tile([P, NT], DT.float16, tag="y")
                        epilogue(
                            ps, 2, mc, yt[:],
                            use_dve_op1=(mode == "fp16" and mc == 3),
                        )
                        nc.sync.dma_start(
                            out=yo[nt - 1, :, mc * NT:(mc + 1) * NT], in_=yt[:]
                        )
                if nt >= 1:
                    h_store.pop(nt - 1)
    nc.compile()
    return nc


def _po2_scale(t: np.ndarray) -> np.float32:
    # match reference._po2_scale bit-for-bit in float32
    thr = np.maximum(np.max(np.abs(t.astype(np.float32))), np.float32(1e-10))
    return np.float32(np.exp2(np.ceil(np.log2(thr))) / np.float32(128.0))


def _fq_weight_int(w: np.ndarray) -> tuple[np.ndarray, np.float32]:
    s = _po2_scale(w)
    q = np.clip(np.round(w.astype(np.float32) / s), -127.0, 127.0)
    return q, s


def _fq_bias(b: np.ndarray) -> np.ndarray:
    s = _po2_scale(b)
    return np.clip(np.round(b.astype(np.float32) / s), -128.0, 127.0) * s


def _pack_x_core(xc: np.ndarray, mode: str) -> np.ndarray:
    """xc [BPC, C, N] f32 -> [NTILES, KC, P, x_cols] device layout."""
    if mode == "hilo":
        xh = xc.astype(BF16)
        xl = (xc - xh.astype(np.float32)).astype(BF16)
        out = np.empty((BPC, NTPB, KC, P, 2 * NT), BF16)
        out[..., :NT] = xh.reshape(BPC, KC, P, NTPB, NT).transpose(0, 3, 1, 2, 4)
        out[..., NT:] = xl.reshape(BPC, KC, P, NTPB, NT).transpose(0, 3, 1, 2, 4)
        return np.ascontiguousarray(out.reshape(NTILES, KC, P, 2 * NT))
    else:
        x16 = xc.astype(np.float16)
        out = x16.reshape(BPC, KC, P, NTPB, NT).transpose(0, 3, 1, 2, 4)
        return np.ascontiguousarray(out.reshape(NTILES, KC, P, NT))


def _run(inputs: dict, trace: bool = False, mode: str | None = None):
    """Shard, run on 8 cores, unshard.  Returns (y_full, BassKernelResults)."""
    mode = mode or MODE
    x = np.asarray(inputs["x"], dtype=np.float32)
    w1 = np.asarray(inputs["w1"], dtype=np.float32)
    b1 = np.asarray(inputs["b1"], dtype=np.float32)
    w2 = np.asarray(inputs["w2"], dtype=np.float32)
    b2 = np.asarray(inputs["b2"], dtype=np.float32)
    s_act = np.float32(np.asarray(inputs["act_scale"], dtype=np.float32).reshape(-1)[0])

    q1, s1 = _fq_weight_int(w1)
    q2, s2 = _fq_weight_int(w2)
    b1q = _fq_bias(b1)
    b2q = _fq_bias(b2)

    w_dt = BF16 if mode == "hilo" else np.float16
    # lhsT layout: [kc, p, o] with contraction index c = kc*P + p
    w1d = np.ascontiguousarray(q1.T.reshape(KC, P, C).astype(w_dt))
    w2d = np.ascontiguousarray(q2.T.reshape(KC, P, C).astype(np.float16))
    # layer-2 bias absorbs the +1024 offset on h': exact, since rowsum(q2)
    # is an integer < 2^13 and s2 is a power of two.
    b2_corr = (
        np.float32(OFF) * q2.sum(axis=1, dtype=np.float64) * np.float64(s2)
    ).astype(np.float32)
    cd = np.empty((P, 10), np.float32)
    cd[:, 0] = np.float32(s1 / s_act)
    cd[:, 1] = np.float32(s2)
    cd[:, 2:6] = ((b1q / s_act).astype(np.float32) + np.float32(OFF)).reshape(MC, P).T
    cd[:, 6:10] = (
        (b2q / s_act).astype(np.float32) - b2_corr + np.float32(OFF)
    ).reshape(MC, P).T

    if mode not in _NC_CACHE:
        _NC_CACHE[mode] = _build_nc(mode)
    nc = _NC_CACHE[mode]

    in_maps = []
    for c in range(NCORES):
        xc = x[c * BPC:(c + 1) * BPC]
        in_maps.append(
            {
                "xp": _pack_x_core(xc, mode),
                "w1d": w1d,
                "w2d": w2d,
                "cd": cd,
            }
        )

    res = run_bass_kernel_spmd(nc, in_maps, core_ids=list(range(NCORES)), trace=trace)

    y = np.empty((B, C, N), np.float32)
    for c in range(NCORES):
        yo = np.asarray(res.results[c]["yo"])  # [NTILES, P, MC*NT] fp16
        yi = (
            (yo.astype(np.float32) - np.float32(OFF))
            .reshape(BPC, NTPB, P, MC, NT)
            .transpose(0, 3, 2, 1, 4)
            .reshape(BPC, C, N)
        )
        y[c * BPC:(c + 1) * BPC] = yi * s_act
    return y, res


def kernel(**inputs) -> np.ndarray:
    y, _ = _run(inputs, trace=False)
    return y
